# revision 9
# baseline (speedup 1.0000x reference)
"""Trainium2 Bass kernel for DynConvLayer (512x512, C=64, K=3, dil=2).

out = where(sd, gelu(conv2(rpad(x_ori))), gelu(dwconv3(rpad(x)))) + x
  x_ori = where(md, gelu(conv1(rpad(x))), x)
  md = 5x5-binary-dilate(mask), sd = mask>0.5, rpad = reflect-pad-2

Sharding: H split across 8 cores (64 rows each, halo 4), W split into 4
panels of 128 cols per core (SBUF capacity). Convs run on the tensor
engine as per-tap [C_in, C_out] matmuls accumulated in PSUM over a
flattened (row*136+col) pixel stream.

conv1+conv3 (fused): a +2-row-shifted copy of x in SBUF partitions
64..127 lets one K=128 matmul cover two taps; the depthwise conv3 rides
in PSUM partitions 64..127 as diagonal weight columns. gelu(conv1) is
written by ACT directly into x_ori, then a predicated copy pulls x back
where ~md (no separate seed DMA).

conv2 (pixel-paired): output rows are processed in blocks of 4 as two
row-pairs (4b,4b+1 | 4b+2,4b+3) living in PSUM partitions 0..63/64..127
of one [128, 272] accumulation. The 3 "diag" matmuls are fully dense
128x128 (each covers 4 tap-applications via the +2-shifted B-half); the
3 "corner" matmuls use an auxiliary +6-shifted pairing tile (X6). This
halves conv2's tensor-engine time versus per-tap M=64 matmuls.

Matmul inputs are bf16 (fp32 PSUM accumulate); the residual +x is added
on the host in fp32. Reflection halos are handled by host padding plus
on-chip strided fix-up copies and per-core edge-mask blends, keeping the
program SPMD-uniform across all 8 cores. DMA traffic is split between
the SP and GpSimd queues.
"""

import os
import sys

import numpy as np

for _p in ("/opt/trn_rl_repo", "/opt/pypackages"):
    if _p not in sys.path:
        sys.path.insert(0, _p)

import concourse.bass as bass
import concourse.bacc as bacc
import concourse.mybir as mybir
from concourse.tile import TileContext
from concourse.bass_utils import run_bass_kernel_spmd

F32 = mybir.dt.float32
BF16 = mybir.dt.bfloat16
U8 = mybir.dt.uint8
AF = mybir.ActivationFunctionType

C = 64
H = W = 512
NCORES = 8
RPC = H // NCORES          # 64 output rows per core
NP = 4                     # W panels per core
PCOL = W // NP             # 128 output cols per panel
PW = PCOL + 8              # 136 slab width (cols -4..132 rel panel)
XROWS = 72                 # x slab rows (-4..68 rel core block)
OROWS = RPC + 4            # 68 rows of x_ori stream (-2..65)
S13 = OROWS * PW           # 9248 conv13 stream length
S2 = RPC * PW              # 8704 conv2 / output stream length
MX = 2                     # front margin of x tile (negative tap offsets)
MO = 2                     # front margin of x_ori tile
MOX = 2                    # front margin of X6 tile
M2 = 2                     # front margin of X2 tile
XF = MX + S13 + 4 * PW + 8     # x tile free size
XOF = MO + S13 + 8             # x_ori/g3 tile free size
X6A = 64 * PW              # X6 A-half extent (x_ori rows -2..61)
X6B = 62 * PW              # X6 B-half extent (s+6PW = rows 4..65)
XF6 = MOX + X6A + 8        # X6 tile free size
XF2 = M2 + 64 * PW + 8     # X2 tile free size
ROWOFF13 = 2 * PW          # conv13 stream -> x tile row offset
CHUNK = 512
NB = RPC // 4              # 16 conv2 blocks of 4 output rows

_CACHE = {}


def _chunks(total):
    out = []
    off = 0
    while off < total:
        n = min(CHUNK, total - off)
        out.append((off, n))
        off += n
    return out


def _build_program(act=None):
    act = AF.Gelu if act is None else act
    key = ("nc", str(act))
    if key in _CACHE:
        return _CACHE[key]
    nc = bacc.Bacc("TRN2", target_bir_lowering=False, debug=False)

    x_in = nc.declare_dram_parameter("x_in", [NP, 64, XROWS * PW], BF16, isOutput=False)
    md_in = nc.declare_dram_parameter("md_in", [NP, 64, S13], U8, isOutput=False)
    sd_in = nc.declare_dram_parameter("sd_in", [NP, 128, NB * 256], U8, isOutput=False)
    w13p_in = nc.declare_dram_parameter("w13p", [128, 3 * 128], BF16, isOutput=False)
    w13s_in = nc.declare_dram_parameter("w13s", [64, 3 * 128], BF16, isOutput=False)
    w2d_in = nc.declare_dram_parameter("w2d", [128, 3 * 128], BF16, isOutput=False)
    w2c_in = nc.declare_dram_parameter("w2c", [128, 3 * 128], BF16, isOutput=False)
    b13_in = nc.declare_dram_parameter("b13", [128, 1], F32, isOutput=False)
    etop_in = nc.declare_dram_parameter("etopm", [64, PW], U8, isOutput=False)
    ebot_in = nc.declare_dram_parameter("ebotm", [64, PW], U8, isOutput=False)
    b2d_in = nc.declare_dram_parameter("b2dup", [128, 1], F32, isOutput=False)
    # bf16 output, rows ordered [even-pairs | odd-pairs]; host reorders
    out_d = nc.declare_dram_parameter("out", [NP, 64, RPC * PCOL], BF16, isOutput=True)

    ch13 = _chunks(S13)

    with TileContext(nc) as tc:
        with (
            tc.tile_pool(name="const", bufs=1) as cpool,
            tc.tile_pool(name="xp", bufs=2) as xpool,
            tc.tile_pool(name="xop", bufs=2) as xopool,
            tc.tile_pool(name="x2p", bufs=2) as x2pool,
            tc.tile_pool(name="x6p", bufs=2) as x6pool,
            tc.tile_pool(name="mp", bufs=1) as mpool,
            tc.tile_pool(name="op", bufs=2) as opool,
            tc.tile_pool(name="ps13", bufs=4, space="PSUM") as ps13pool,
        ):
            w13pt = cpool.tile([128, 3 * 128], BF16, name="w13pt")
            w13st = cpool.tile([64, 3 * 128], BF16, name="w13st")
            w2dt = cpool.tile([128, 3 * 128], BF16, name="w2dt")
            w2ct = cpool.tile([128, 3 * 128], BF16, name="w2ct")
            b13t = cpool.tile([128, 1], F32, name="b13t")
            b2t = cpool.tile([128, 1], F32, name="b2t")
            etopt = cpool.tile([64, PW], U8, name="etopt")
            ebott = cpool.tile([64, PW], U8, name="ebott")
            # consts ride the scalar queue so panel-0 x bands own sync/gpsimd
            for t, d in (
                (w13pt, w13p_in), (w13st, w13s_in), (b13t, b13_in),
                (etopt, etop_in), (ebott, ebot_in), (w2dt, w2d_in),
                (w2ct, w2c_in), (b2t, b2d_in),
            ):
                nc.scalar.dma_start(out=t[:, :], in_=d.ap())

            P = [None] * NP  # per-panel tile/state records

            def setup_panel(p):
                s = {}
                s["xt"] = xpool.tile([128, XF], BF16, name=f"xt{p}", tag="xt")
                s["xori"] = xopool.tile([128, XOF], BF16, name=f"xori{p}", tag="xori")
                s["x2t"] = x2pool.tile([128, XF2], BF16, name=f"x2_{p}", tag="x2")
                s["x6t"] = x6pool.tile([128, XF6], BF16, name=f"x6_{p}", tag="x6")
                s["mdt"] = mpool.tile([64, S13], U8, name=f"mdt{p}", tag="mdt")
                s["outt"] = opool.tile([128, NB * 256], BF16, name=f"outt{p}", tag="outt")
                s["g3e"] = opool.tile([128, 32 * PW], BF16, name=f"g3e_{p}", tag="g3e")
                xt, x2t, x6t = s["xt"], s["x2t"], s["x6t"]
                # margins: read by garbage output positions, never used
                nc.vector.memset(xt[0:128, 0:MX], 0.0)
                nc.vector.memset(xt[0:128, MX + XROWS * PW: XF], 0.0)
                nc.vector.memset(x2t[0:128, 0:M2], 0.0)
                nc.vector.memset(x2t[0:128, M2 + 64 * PW: XF2], 0.0)
                nc.vector.memset(x6t[0:128, 0:MOX], 0.0)
                nc.vector.memset(x6t[0:128, MOX + X6A: XF6], 0.0)
                nc.vector.memset(x6t[64:128, MOX + X6B: MOX + X6A], 0.0)
                # x row-band DMAs (A-half only), alternating queues; the
                # +2-row B-half is built on-chip on the scalar queue
                bands = ((0, 12), (12, 34), (34, 56), (56, XROWS))
                for bi, (r0, r1) in enumerate(bands):
                    eng = nc.gpsimd if bi % 2 == 0 else nc.sync
                    eng.dma_start(
                        out=xt[0:64, MX + r0 * PW: MX + r1 * PW],
                        in_=x_in.ap()[p, 0:64, r0 * PW: r1 * PW],
                    )
                # B rows [b0, b1) = A rows +2, emitted once the covering A
                # band has been requested (tile deps order the transfers)
                for bi, (b0, b1) in enumerate(((0, 10), (10, 32), (32, 54), (54, 70))):
                    eng = nc.gpsimd if bi % 2 == 0 else nc.sync
                    eng.dma_start(
                        out=xt[64:128, MX + b0 * PW: MX + b1 * PW],
                        in_=xt[0:64, MX + (b0 + 2) * PW: MX + (b1 + 2) * PW],
                    )
                s["state"] = dict(done=0, a2=0, b2=0, a6=0, b6=0, g3=0)
                P[p] = s

            def emit_masks(p):
                nc.scalar.dma_start(out=P[p]["mdt"][0:64, 0:S13], in_=md_in.ap()[p])

            def emit_sd(p):
                sdt = mpool.tile([128, NB * 256], U8, name=f"sdt{p}", tag="sdt")
                P[p]["sdt"] = sdt
                nc.scalar.dma_start(out=sdt[0:128, :], in_=sd_in.ap()[p])

            def c13_group(p, c0, c1):
                s = P[p]
                xt, xori, x2t, x6t, mdt = s["xt"], s["xori"], s["x2t"], s["x6t"], s["mdt"]
                st = s["state"]
                xov = xori[0:64, MO: MO + S13].rearrange("a (r c) -> a r c", c=PW)
                grp = ch13[c0:c1]
                # two 512-col chunks share one [128, 1024] two-bank PSUM
                # superslot -> one ACT + one blend per 1024 cols
                pst = [
                    ps13pool.tile([128, 2 * CHUNK], F32,
                                  name=f"ps13_{p}_{c0 + k}", tag="ps13")
                    for k in range((len(grp) + 1) // 2)
                ]
                for wdx in range(6):
                    if wdx < 3:  # tap pair (-2,dc)+(0,dc), K=128
                        dc = 2 * (wdx - 1)
                        lhs = w13pt[0:128, 128 * wdx: 128 * (wdx + 1)]
                        for k, (o, n) in enumerate(grp):
                            c0 = CHUNK * (k % 2)
                            nc.tensor.matmul(
                                pst[k // 2][0:128, c0: c0 + n],
                                lhs,
                                xt[0:128, MX + o + dc: MX + o + dc + n],
                                start=(wdx == 0), stop=(wdx == 5),
                            )
                    else:  # single tap (2,dc), K=64
                        dc = 2 * (wdx - 4)
                        lhs = w13st[0:64, 128 * (wdx - 3): 128 * (wdx - 2)]
                        for k, (o, n) in enumerate(grp):
                            off = MX + o + 4 * PW + dc
                            c0 = CHUNK * (k % 2)
                            nc.tensor.matmul(
                                pst[k // 2][0:128, c0: c0 + n],
                                lhs,
                                xt[0:64, off: off + n],
                                start=(wdx == 0), stop=(wdx == 5),
                            )
                for k2 in range((len(grp) + 1) // 2):
                    pair = grp[2 * k2: 2 * k2 + 2]
                    o = pair[0][0]
                    n = sum(x[1] for x in pair)
                    # one ACT for both halves: gelu(conv1)+b1 -> x_ori
                    # (parts 0-63), gelu(conv3)+b3 -> g3 halo-stream
                    # (parts 64-127); then pull x where ~md
                    nc.scalar.activation(
                        xori[0:128, MO + o: MO + o + n], pst[k2][0:128, 0:n],
                        act, bias=b13t[0:128, 0:1],
                    )
                    nc.vector.copy_predicated(
                        xori[0:64, MO + o: MO + o + n],
                        mdt[0:64, o: o + n],
                        xt[0:64, MX + o + 2 * PW: MX + o + 2 * PW + n],
                    )
                # -- group epilogue over fully-predicated rows
                o_end = grp[-1][0] + grp[-1][1]
                last = o_end >= S13
                done = OROWS if last else o_end // PW
                done_prev = st["done"]
                if done > done_prev:
                    if p == 0:
                        for dst, src in ((2, 6), (3, 5)):
                            nc.vector.tensor_copy(
                                xov[:, done_prev:done, dst: dst + 1],
                                xov[:, done_prev:done, src: src + 1],
                            )
                    if p == NP - 1:
                        for dst, src in ((132, 130), (133, 129)):
                            nc.vector.tensor_copy(
                                xov[:, done_prev:done, dst: dst + 1],
                                xov[:, done_prev:done, src: src + 1],
                            )
                if done_prev < 5 <= done:
                    # top reflect blend (rows -2,-1 <- 2,1), cores 0/7 only
                    for dst, src in ((0, 4), (1, 3)):
                        nc.vector.copy_predicated(
                            xori[0:64, MO + dst * PW: MO + (dst + 1) * PW],
                            etopt[0:64, 0:PW],
                            xori[0:64, MO + src * PW: MO + (src + 1) * PW],
                        )
                if last:
                    for dst, src in ((OROWS - 2, OROWS - 4), (OROWS - 1, OROWS - 5)):
                        nc.vector.copy_predicated(
                            xori[0:64, MO + dst * PW: MO + (dst + 1) * PW],
                            ebott[0:64, 0:PW],
                            xori[0:64, MO + src * PW: MO + (src + 1) * PW],
                        )
                # X2 pieces (conv2 diag pairing): A-half = x_ori stream rows
                # 2..65 (x_ori rows 0..63), B-half = A shifted +2 rows.
                a2_hi = 64 if last else max(0, min(done - 2, 64))
                if a2_hi > st["a2"]:
                    nc.sync.dma_start(
                        out=x2t[0:64, M2 + st["a2"] * PW: M2 + a2_hi * PW],
                        in_=xori[0:64, MO + (st["a2"] + 2) * PW: MO + (a2_hi + 2) * PW],
                    )
                    st["a2"] = a2_hi
                b2_hi = 64 if last else max(0, min(done - 4, 62))
                if b2_hi > st["b2"]:
                    nc.sync.dma_start(
                        out=x2t[64:128, M2 + st["b2"] * PW: M2 + b2_hi * PW],
                        in_=xori[0:64, MO + (st["b2"] + 4) * PW: MO + (b2_hi + 4) * PW],
                    )
                    st["b2"] = b2_hi
                # g3 duplicates packed 2 rows per 4-row block: even pairs in
                # parts 0:64, odd pairs in parts 64:128 -> one [128, .] blend
                g3_hi = min((o_end - 2 * PW) // (4 * PW), NB) if not last else NB
                if g3_hi > st["g3"]:
                    g3src = xori[64:128, MO + 2 * PW: MO + 2 * PW + S2].rearrange(
                        "a (b w) -> a b w", w=4 * PW)
                    nc.sync.dma_start(
                        out=P[p]["g3e"][0:64, st["g3"] * 2 * PW: g3_hi * 2 * PW].rearrange(
                            "a (b w) -> a b w", w=2 * PW),
                        in_=g3src[:, st["g3"]: g3_hi, 0: 2 * PW],
                    )
                    nc.gpsimd.dma_start(
                        out=P[p]["g3e"][64:128, st["g3"] * 2 * PW: g3_hi * 2 * PW].rearrange(
                            "a (b w) -> a b w", w=2 * PW),
                        in_=g3src[:, st["g3"]: g3_hi, 2 * PW: 4 * PW],
                    )
                    st["g3"] = g3_hi
                # X6 pieces (conv2 corner pairing): A-half = x_ori stream rows
                # 0..63 (x_ori rows -2..61), B-half = A shifted +6 rows.
                # Gate on top blend (done>=5) and bottom blend (last).
                if done >= 5:
                    a6_hi = 64 if last else max(0, min(done - 2, 64))
                    if a6_hi > st["a6"]:
                        nc.gpsimd.dma_start(
                            out=x6t[0:64, MOX + st["a6"] * PW: MOX + a6_hi * PW],
                            in_=xori[0:64, MO + st["a6"] * PW: MO + a6_hi * PW],
                        )
                        st["a6"] = a6_hi
                    b6_hi = 62 if last else max(0, min(done - 8, 62))
                    if b6_hi > st["b6"]:
                        nc.gpsimd.dma_start(
                            out=x6t[64:128, MOX + st["b6"] * PW: MOX + b6_hi * PW],
                            in_=xori[0:64, MO + (st["b6"] + 6) * PW: MO + (b6_hi + 6) * PW],
                        )
                        st["b6"] = b6_hi
                st["done"] = done

            def conv2_block(p, b):
                s = P[p]
                xori, x2t, x6t, outt, g3e = s["xori"], s["x2t"], s["x6t"], s["outt"], s["g3e"]
                sdt = s["sdt"]
                ps = ps13pool.tile([128, CHUNK], F32, name=f"ps2_{p}_{b}", tag="ps13")
                for kc in range(3):
                    dc = 2 * (kc - 1)
                    nc.tensor.matmul(
                        ps[0:128, 0:272],
                        w2dt[0:128, 128 * kc: 128 * (kc + 1)],
                        x2t[0:128, M2 + 4 * b * PW + dc: M2 + 4 * b * PW + dc + 272],
                        start=(kc == 0), stop=False,
                    )
                for kc in range(3):
                    dc = 2 * (kc - 1)
                    nc.tensor.matmul(
                        ps[0:128, 0:272],
                        w2ct[0:128, 128 * kc: 128 * (kc + 1)],
                        x6t[0:128, MOX + 4 * b * PW + dc: MOX + 4 * b * PW + dc + 272],
                        start=False, stop=(kc == 2),
                    )
                # gelu(conv2 + b2) for all 4 rows in one ACT, packed into the
                # interleaved output tile; then pull gelu3 where ~sd.
                ob = outt[0:128, 256 * b: 256 * (b + 1)].rearrange(
                    "a (r c) -> a r c", c=128)
                nc.scalar.activation(
                    ob,
                    ps[0:128, 0:272].rearrange(
                        "a (r c) -> a r c", c=PW)[:, :, 4:132],
                    act, bias=b2t[0:128, 0:1],
                )
                sb = sdt[0:128, 256 * b: 256 * (b + 1)].rearrange(
                    "a (r c) -> a r c", c=128)
                # both row-pair halves come from the packed g3 duplicate
                g3v = g3e[0:128, 0:32 * PW].rearrange("a (r c) -> a r c", c=PW)
                nc.vector.copy_predicated(
                    ob, sb, g3v[:, 2 * b: 2 * b + 2, 4:132],
                )
                if b % 4 == 3:
                    g = b // 4
                    nc.scalar.dma_start(
                        out=out_d.ap()[p, 0:64, 1024 * g: 1024 * (g + 1)],
                        in_=outt[0:64, 1024 * g: 1024 * (g + 1)],
                    )
                    nc.scalar.dma_start(
                        out=out_d.ap()[p, 0:64, 4096 + 1024 * g: 4096 + 1024 * (g + 1)],
                        in_=outt[64:128, 1024 * g: 1024 * (g + 1)],
                    )

            # software-pipelined driver: conv2 blocks trail their panel's
            # conv13 by one piece-gated group, so the PE never waits for the
            # full-panel pairing tiles; prev-panel late blocks fill the
            # conv13 ramp of the next panel
            setup_panel(0)
            emit_masks(0)
            emit_sd(0)
            for p in range(NP):
                if p == 0:
                    c13_group(0, 0, 2)
                    c13_group(0, 2, 8)
                else:
                    c13_group(p, 0, 8)
                if p > 0:
                    for b in range(9, NB):
                        conv2_block(p - 1, b)
                    emit_sd(p)
                if p + 1 < NP:
                    setup_panel(p + 1)
                c13_group(p, 8, 16)
                for b in range(0, 6):
                    conv2_block(p, b)
                c13_group(p, 16, 19)
                if p + 1 < NP:
                    emit_masks(p + 1)
                for b in range(6, 9):
                    conv2_block(p, b)
            for b in range(9, NB):
                conv2_block(NP - 1, b)

    nc.compile()
    _CACHE[key] = nc
    return nc


def _pack_weights(w1, w2, w3, b1, b2, b3):
    w13p = np.zeros((128, 3, 128), np.float32)
    w13s = np.zeros((64, 3, 128), np.float32)
    w2d = np.zeros((128, 3, 128), np.float32)
    w2c = np.zeros((128, 3, 128), np.float32)
    di = np.arange(64)
    for k in range(3):
        w13p[0:64, k, 0:64] = w1[:, :, 0, k].T
        w13p[64:128, k, 0:64] = w1[:, :, 1, k].T
        w13p[di, k, 64 + di] = w3[:, 0, 0, k]
        w13p[64 + di, k, 64 + di] = w3[:, 0, 1, k]

        w13s[0:64, k, 0:64] = w1[:, :, 2, k].T
        w13s[di, k, 64 + di] = w3[:, 0, 2, k]

        # conv2 pixel-pair: dense diag matmul covers 4 tap-applications
        w2d[0:64, k, 0:64] = w2[:, :, 1, k].T     # x(4b..) -> out(4b..)
        w2d[0:64, k, 64:128] = w2[:, :, 0, k].T   # x(4b..) -> out(4b+2..)
        w2d[64:128, k, 0:64] = w2[:, :, 2, k].T   # x(4b+2..) -> out(4b..)
        w2d[64:128, k, 64:128] = w2[:, :, 1, k].T
        # corner matmul via X6 (+6-shift pairing)
        w2c[0:64, k, 0:64] = w2[:, :, 0, k].T     # x(4b-2..) -> out(4b..)
        w2c[64:128, k, 64:128] = w2[:, :, 2, k].T  # x(4b+4..) -> out(4b+2..)
    return (
        np.ascontiguousarray(w13p.reshape(128, 384)),
        np.ascontiguousarray(w13s.reshape(64, 384)),
        np.ascontiguousarray(w2d.reshape(128, 384)),
        np.ascontiguousarray(w2c.reshape(128, 384)),
        np.concatenate([b1, b3]).reshape(128, 1).astype(np.float32),
        np.concatenate([b2, b2]).reshape(128, 1).astype(np.float32),
    )


def _dilate5(m):
    # 5x5 binary dilation, SAME/zero-pad semantics (max-pool)
    hh, ww = m.shape
    mp = np.pad(m, 2)
    a = np.maximum.reduce([mp[k: k + hh] for k in range(5)])      # [hh, ww+4]
    return np.maximum.reduce([a[:, k: k + ww] for k in range(5)])  # [hh, ww]


def make_in_maps(x, mask, w1, b1, w2, b2, w3, b3):
    import ml_dtypes
    BF = ml_dtypes.bfloat16
    x = np.asarray(x, np.float32)
    mask = np.asarray(mask, np.float32)

    w13p, w13s, w2d, w2c, b13p, b2dp = _pack_weights(
        np.asarray(w1, np.float32), np.asarray(w2, np.float32),
        np.asarray(w3, np.float32), np.asarray(b1, np.float32),
        np.asarray(b2, np.float32), np.asarray(b3, np.float32))
    w13p = w13p.astype(BF); w13s = w13s.astype(BF)
    w2d = w2d.astype(BF); w2c = w2c.astype(BF)

    xp32 = np.pad(x[0], ((0, 0), (4, 6), (4, 4)), mode="reflect")  # [64,522,520]
    xp = xp32.astype(BF)
    m = mask[0, 0]
    # inverted dilated mask: blend pulls x where ~md
    mdi = 1 - (_dilate5(m) > 0.5).astype(np.uint8)
    mdp = np.pad(mdi, ((2, 2), (4, 4)), mode="edge")   # [516,520]
    sdi = 1 - (m > 0.5).astype(np.uint8)               # inverted: pull gelu3

    ones = np.ones((64, PW), np.uint8)
    zeros = np.zeros((64, PW), np.uint8)

    in_maps = []
    for i in range(NCORES):
        r0 = RPC * i
        xc = np.empty((NP, 64, XROWS, PW), BF)
        mdc = np.empty((NP, 64, OROWS, PW), np.uint8)
        sdc = np.empty((NP, 128, NB, 2, PCOL), np.uint8)
        for p in range(NP):
            c0 = PCOL * p
            xc[p] = xp[:, r0: r0 + XROWS, c0: c0 + PW]
            mdc[p] = np.broadcast_to(
                mdp[r0: r0 + OROWS, c0: c0 + PW], (64, OROWS, PW))
            # packed interleaved ~sd: parts 0-63 = rows (4b,4b+1),
            # parts 64-127 = rows (4b+2,4b+3)
            sdi4 = sdi[r0: r0 + RPC, c0: c0 + PCOL].reshape(NB, 2, 2, PCOL)
            sdc[p, 0:64] = np.broadcast_to(sdi4[:, 0], (64, NB, 2, PCOL))
            sdc[p, 64:128] = np.broadcast_to(sdi4[:, 1], (64, NB, 2, PCOL))
        xc = np.ascontiguousarray(xc).reshape(NP, 64, XROWS * PW)
        mdc = np.ascontiguousarray(mdc).reshape(NP, 64, S13)
        sdc = np.ascontiguousarray(sdc).reshape(NP, 128, NB * 256)
        in_maps.append({
            "x_in": xc, "md_in": mdc, "sd_in": sdc,
            "w13p": w13p, "w13s": w13s, "w2d": w2d, "w2c": w2c,
            "b13": b13p, "b2dup": b2dp,
            "etopm": ones if i == 0 else zeros,
            "ebotm": ones if i == NCORES - 1 else zeros,
        })

    return in_maps


def kernel(x, mask, w1, b1, w2, b2, w3, b3):
    nc = _build_program()
    in_maps = make_in_maps(x, mask, w1, b1, w2, b2, w3, b3)
    global _last_in_maps
    _last_in_maps = in_maps
    res = run_bass_kernel_spmd(nc, in_maps, list(range(NCORES)))
    out = np.empty((1, C, H, W), np.float32)
    for i in range(NCORES):
        o = np.asarray(res.results[i]["out"], np.float32)  # [NP, 64, 8192] bf16
        # rows ordered [even-pairs | odd-pairs]: (half, b, rr) -> 4b+2*half+rr
        o = o.reshape(NP, C, 2, NB, 2, PCOL).transpose(1, 3, 2, 4, 0, 5)
        out[0, :, RPC * i: RPC * (i + 1), :] = o.reshape(C, RPC, W)
    out += np.asarray(x, np.float32).reshape(1, C, H, W)
    return out



# revision 10
# speedup vs baseline: 1.0207x; 1.0207x over previous
"""Trainium2 Bass kernel for DynConvLayer (512x512, C=64, K=3, dil=2).

out = where(sd, gelu(conv2(rpad(x_ori))), gelu(dwconv3(rpad(x)))) + x
  x_ori = where(md, gelu(conv1(rpad(x))), x)
  md = 5x5-binary-dilate(mask), sd = mask>0.5, rpad = reflect-pad-2

Sharding: H split across 8 cores (64 rows each, halo 4), W split into 4
panels of 128 cols per core (SBUF capacity). Convs run on the tensor
engine as per-tap [C_in, C_out] matmuls accumulated in PSUM over a
flattened (row*136+col) pixel stream.

conv1+conv3 (fused): a +2-row-shifted copy of x in SBUF partitions
64..127 lets one K=128 matmul cover two taps; the depthwise conv3 rides
in PSUM partitions 64..127 as diagonal weight columns. gelu(conv1) is
written by ACT directly into x_ori, then a predicated copy pulls x back
where ~md (no separate seed DMA).

conv2 (pixel-paired): output rows are processed in blocks of 4 as two
row-pairs (4b,4b+1 | 4b+2,4b+3) living in PSUM partitions 0..63/64..127
of one [128, 272] accumulation. The 3 "diag" matmuls are fully dense
128x128 (each covers 4 tap-applications via the +2-shifted B-half); the
3 "corner" matmuls use an auxiliary +6-shifted pairing tile (X6). This
halves conv2's tensor-engine time versus per-tap M=64 matmuls.

Matmul inputs are bf16 (fp32 PSUM accumulate); the residual +x is added
on the host in fp32. Reflection halos are handled by host padding plus
on-chip strided fix-up copies and per-core edge-mask blends, keeping the
program SPMD-uniform across all 8 cores. DMA traffic is split between
the SP and GpSimd queues.
"""

import os
import sys

import numpy as np

for _p in ("/opt/trn_rl_repo", "/opt/pypackages"):
    if _p not in sys.path:
        sys.path.insert(0, _p)

import concourse.bass as bass
import concourse.bacc as bacc
import concourse.mybir as mybir
from concourse.tile import TileContext
from concourse.bass_utils import run_bass_kernel_spmd

F32 = mybir.dt.float32
BF16 = mybir.dt.bfloat16
U8 = mybir.dt.uint8
AF = mybir.ActivationFunctionType

C = 64
H = W = 512
NCORES = 8
RPC = H // NCORES          # 64 output rows per core
NP = 4                     # W panels per core
PCOL = W // NP             # 128 output cols per panel
PW = PCOL + 8              # 136 slab width (cols -4..132 rel panel)
XROWS = 72                 # x slab rows (-4..68 rel core block)
OROWS = RPC + 4            # 68 rows of x_ori stream (-2..65)
S13 = OROWS * PW           # 9248 conv13 stream length
S2 = RPC * PW              # 8704 conv2 / output stream length
MX = 2                     # front margin of x tile (negative tap offsets)
MO = 2                     # front margin of x_ori tile
MOX = 2                    # front margin of X6 tile
M2 = 2                     # front margin of X2 tile
XF = MX + S13 + 4 * PW + 8     # x tile free size
XOF = MO + S13 + 8             # x_ori/g3 tile free size
X6A = 64 * PW              # X6 A-half extent (x_ori rows -2..61)
X6B = 62 * PW              # X6 B-half extent (s+6PW = rows 4..65)
XF6 = MOX + X6A + 8        # X6 tile free size
XF2 = M2 + 64 * PW + 8     # X2 tile free size
ROWOFF13 = 2 * PW          # conv13 stream -> x tile row offset
CHUNK = 512
NB = RPC // 4              # 16 conv2 blocks of 4 output rows

_CACHE = {}


def _chunks(total):
    out = []
    off = 0
    while off < total:
        n = min(CHUNK, total - off)
        out.append((off, n))
        off += n
    return out


def _build_program(act=None):
    act = AF.Gelu if act is None else act
    key = ("nc", str(act))
    if key in _CACHE:
        return _CACHE[key]
    nc = bacc.Bacc("TRN2", target_bir_lowering=False, debug=False)

    x_in = nc.declare_dram_parameter("x_in", [NP, 64, XROWS * PW], BF16, isOutput=False)
    md_in = nc.declare_dram_parameter("md_in", [NP, 64, S13], U8, isOutput=False)
    sd_in = nc.declare_dram_parameter("sd_in", [NP, 128, NB * 256], U8, isOutput=False)
    w13p_in = nc.declare_dram_parameter("w13p", [128, 3 * 128], BF16, isOutput=False)
    w13s_in = nc.declare_dram_parameter("w13s", [64, 3 * 128], BF16, isOutput=False)
    w2d_in = nc.declare_dram_parameter("w2d", [128, 3 * 128], BF16, isOutput=False)
    w2c_in = nc.declare_dram_parameter("w2c", [128, 3 * 128], BF16, isOutput=False)
    b13_in = nc.declare_dram_parameter("b13", [128, 1], F32, isOutput=False)
    etop_in = nc.declare_dram_parameter("etopm", [64, PW], U8, isOutput=False)
    ebot_in = nc.declare_dram_parameter("ebotm", [64, PW], U8, isOutput=False)
    b2d_in = nc.declare_dram_parameter("b2dup", [128, 1], F32, isOutput=False)
    # bf16 output, rows ordered [even-pairs | odd-pairs]; host reorders
    out_d = nc.declare_dram_parameter("out", [NP, 64, RPC * PCOL], BF16, isOutput=True)

    ch13 = _chunks(S13)

    with TileContext(nc) as tc:
        with (
            tc.tile_pool(name="const", bufs=1) as cpool,
            tc.tile_pool(name="xp", bufs=2) as xpool,
            tc.tile_pool(name="xop", bufs=2) as xopool,
            tc.tile_pool(name="x2p", bufs=2) as x2pool,
            tc.tile_pool(name="x6p", bufs=2) as x6pool,
            tc.tile_pool(name="mp", bufs=1) as mpool,
            tc.tile_pool(name="op", bufs=2) as opool,
            tc.tile_pool(name="ps13", bufs=4, space="PSUM") as ps13pool,
        ):
            w13pt = cpool.tile([128, 3 * 128], BF16, name="w13pt")
            w13st = cpool.tile([64, 3 * 128], BF16, name="w13st")
            w2dt = cpool.tile([128, 3 * 128], BF16, name="w2dt")
            w2ct = cpool.tile([128, 3 * 128], BF16, name="w2ct")
            b13t = cpool.tile([128, 1], F32, name="b13t")
            b2t = cpool.tile([128, 1], F32, name="b2t")
            etopt = cpool.tile([64, PW], U8, name="etopt")
            ebott = cpool.tile([64, PW], U8, name="ebott")
            # consts ride the scalar queue so panel-0 x bands own sync/gpsimd
            for t, d in (
                (w13pt, w13p_in), (w13st, w13s_in), (b13t, b13_in),
                (etopt, etop_in), (ebott, ebot_in), (w2dt, w2d_in),
                (w2ct, w2c_in), (b2t, b2d_in),
            ):
                nc.scalar.dma_start(out=t[:, :], in_=d.ap())

            P = [None] * NP  # per-panel tile/state records

            def setup_panel(p):
                s = {}
                s["xt"] = xpool.tile([128, XF], BF16, name=f"xt{p}", tag="xt")
                s["xori"] = xopool.tile([128, XOF], BF16, name=f"xori{p}", tag="xori")
                s["x2t"] = x2pool.tile([128, XF2], BF16, name=f"x2_{p}", tag="x2")
                s["x6t"] = x6pool.tile([128, XF6], BF16, name=f"x6_{p}", tag="x6")
                s["mdt"] = mpool.tile([64, S13], U8, name=f"mdt{p}", tag="mdt")
                s["outt"] = opool.tile([128, NB * 256], BF16, name=f"outt{p}", tag="outt")
                s["g3e"] = opool.tile([128, 32 * PW], BF16, name=f"g3e_{p}", tag="g3e")
                xt, x2t, x6t = s["xt"], s["x2t"], s["x6t"]
                # margins: read by garbage output positions, never used
                nc.vector.memset(xt[0:128, 0:MX], 0.0)
                nc.vector.memset(xt[0:128, MX + XROWS * PW: XF], 0.0)
                nc.vector.memset(x2t[0:128, 0:M2], 0.0)
                nc.vector.memset(x2t[0:128, M2 + 64 * PW: XF2], 0.0)
                nc.vector.memset(x6t[0:128, 0:MOX], 0.0)
                nc.vector.memset(x6t[0:128, MOX + X6A: XF6], 0.0)
                nc.vector.memset(x6t[64:128, MOX + X6B: MOX + X6A], 0.0)
                # x row-band DMAs (A-half only), alternating queues; the
                # +2-row B-half is built on-chip on the scalar queue
                bands = ((0, 12), (12, 34), (34, 56), (56, XROWS))
                for bi, (r0, r1) in enumerate(bands):
                    eng = nc.gpsimd if bi % 2 == 0 else nc.sync
                    eng.dma_start(
                        out=xt[0:64, MX + r0 * PW: MX + r1 * PW],
                        in_=x_in.ap()[p, 0:64, r0 * PW: r1 * PW],
                    )
                # B rows [b0, b1) = A rows +2, emitted once the covering A
                # band has been requested (tile deps order the transfers)
                for bi, (b0, b1) in enumerate(((0, 10), (10, 32), (32, 54), (54, 70))):
                    eng = nc.gpsimd if bi % 2 == 0 else nc.sync
                    eng.dma_start(
                        out=xt[64:128, MX + b0 * PW: MX + b1 * PW],
                        in_=xt[0:64, MX + (b0 + 2) * PW: MX + (b1 + 2) * PW],
                    )
                s["state"] = dict(done=0, a2=0, b2=0, a6=0, b6=0, g3=0)
                P[p] = s

            def emit_masks(p):
                nc.gpsimd.dma_start(out=P[p]["mdt"][0:64, 0:S13], in_=md_in.ap()[p])

            def emit_sd(p):
                sdt = mpool.tile([128, NB * 256], U8, name=f"sdt{p}", tag="sdt")
                P[p]["sdt"] = sdt
                nc.gpsimd.dma_start(out=sdt[0:128, :], in_=sd_in.ap()[p])

            def c13_group(p, c0, c1):
                s = P[p]
                xt, xori, x2t, x6t, mdt = s["xt"], s["xori"], s["x2t"], s["x6t"], s["mdt"]
                st = s["state"]
                xov = xori[0:64, MO: MO + S13].rearrange("a (r c) -> a r c", c=PW)
                grp = ch13[c0:c1]
                # two 512-col chunks share one [128, 1024] two-bank PSUM
                # superslot -> one ACT + one blend per 1024 cols
                pst = [
                    ps13pool.tile([128, 2 * CHUNK], F32,
                                  name=f"ps13_{p}_{c0 + k}", tag="ps13")
                    for k in range((len(grp) + 1) // 2)
                ]
                for wdx in range(6):
                    if wdx < 3:  # tap pair (-2,dc)+(0,dc), K=128
                        dc = 2 * (wdx - 1)
                        lhs = w13pt[0:128, 128 * wdx: 128 * (wdx + 1)]
                        for k, (o, n) in enumerate(grp):
                            c0 = CHUNK * (k % 2)
                            nc.tensor.matmul(
                                pst[k // 2][0:128, c0: c0 + n],
                                lhs,
                                xt[0:128, MX + o + dc: MX + o + dc + n],
                                start=(wdx == 0), stop=(wdx == 5),
                            )
                    else:  # single tap (2,dc), K=64
                        dc = 2 * (wdx - 4)
                        lhs = w13st[0:64, 128 * (wdx - 3): 128 * (wdx - 2)]
                        for k, (o, n) in enumerate(grp):
                            off = MX + o + 4 * PW + dc
                            c0 = CHUNK * (k % 2)
                            nc.tensor.matmul(
                                pst[k // 2][0:128, c0: c0 + n],
                                lhs,
                                xt[0:64, off: off + n],
                                start=(wdx == 0), stop=(wdx == 5),
                            )
                for k2 in range((len(grp) + 1) // 2):
                    pair = grp[2 * k2: 2 * k2 + 2]
                    o = pair[0][0]
                    n = sum(x[1] for x in pair)
                    # one ACT for both halves: gelu(conv1)+b1 -> x_ori
                    # (parts 0-63), gelu(conv3)+b3 -> g3 halo-stream
                    # (parts 64-127); then pull x where ~md
                    nc.scalar.activation(
                        xori[0:128, MO + o: MO + o + n], pst[k2][0:128, 0:n],
                        act, bias=b13t[0:128, 0:1],
                    )
                    nc.vector.copy_predicated(
                        xori[0:64, MO + o: MO + o + n],
                        mdt[0:64, o: o + n],
                        xt[0:64, MX + o + 2 * PW: MX + o + 2 * PW + n],
                    )
                # -- group epilogue over fully-predicated rows
                o_end = grp[-1][0] + grp[-1][1]
                last = o_end >= S13
                done = OROWS if last else o_end // PW
                done_prev = st["done"]
                if done > done_prev:
                    if p == 0:
                        for dst, src in ((2, 6), (3, 5)):
                            nc.vector.tensor_copy(
                                xov[:, done_prev:done, dst: dst + 1],
                                xov[:, done_prev:done, src: src + 1],
                            )
                    if p == NP - 1:
                        for dst, src in ((132, 130), (133, 129)):
                            nc.vector.tensor_copy(
                                xov[:, done_prev:done, dst: dst + 1],
                                xov[:, done_prev:done, src: src + 1],
                            )
                if done_prev < 5 <= done:
                    # top reflect blend (rows -2,-1 <- 2,1), cores 0/7 only
                    for dst, src in ((0, 4), (1, 3)):
                        nc.vector.copy_predicated(
                            xori[0:64, MO + dst * PW: MO + (dst + 1) * PW],
                            etopt[0:64, 0:PW],
                            xori[0:64, MO + src * PW: MO + (src + 1) * PW],
                        )
                if last:
                    for dst, src in ((OROWS - 2, OROWS - 4), (OROWS - 1, OROWS - 5)):
                        nc.vector.copy_predicated(
                            xori[0:64, MO + dst * PW: MO + (dst + 1) * PW],
                            ebott[0:64, 0:PW],
                            xori[0:64, MO + src * PW: MO + (src + 1) * PW],
                        )
                # X2 pieces (conv2 diag pairing): A-half = x_ori stream rows
                # 2..65 (x_ori rows 0..63), B-half = A shifted +2 rows.
                a2_hi = 64 if last else max(0, min(done - 2, 64))
                if a2_hi > st["a2"]:
                    nc.sync.dma_start(
                        out=x2t[0:64, M2 + st["a2"] * PW: M2 + a2_hi * PW],
                        in_=xori[0:64, MO + (st["a2"] + 2) * PW: MO + (a2_hi + 2) * PW],
                    )
                    st["a2"] = a2_hi
                b2_hi = 64 if last else max(0, min(done - 4, 62))
                if b2_hi > st["b2"]:
                    nc.sync.dma_start(
                        out=x2t[64:128, M2 + st["b2"] * PW: M2 + b2_hi * PW],
                        in_=xori[0:64, MO + (st["b2"] + 4) * PW: MO + (b2_hi + 4) * PW],
                    )
                    st["b2"] = b2_hi
                # g3 duplicates packed 2 rows per 4-row block: even pairs in
                # parts 0:64, odd pairs in parts 64:128 -> one [128, .] blend
                g3_hi = min((o_end - 2 * PW) // (4 * PW), NB) if not last else NB
                if g3_hi > st["g3"]:
                    g3src = xori[64:128, MO + 2 * PW: MO + 2 * PW + S2].rearrange(
                        "a (b w) -> a b w", w=4 * PW)
                    nc.sync.dma_start(
                        out=P[p]["g3e"][0:64, st["g3"] * 2 * PW: g3_hi * 2 * PW].rearrange(
                            "a (b w) -> a b w", w=2 * PW),
                        in_=g3src[:, st["g3"]: g3_hi, 0: 2 * PW],
                    )
                    nc.gpsimd.dma_start(
                        out=P[p]["g3e"][64:128, st["g3"] * 2 * PW: g3_hi * 2 * PW].rearrange(
                            "a (b w) -> a b w", w=2 * PW),
                        in_=g3src[:, st["g3"]: g3_hi, 2 * PW: 4 * PW],
                    )
                    st["g3"] = g3_hi
                # X6 pieces (conv2 corner pairing): A-half = x_ori stream rows
                # 0..63 (x_ori rows -2..61), B-half = A shifted +6 rows.
                # Gate on top blend (done>=5) and bottom blend (last).
                if done >= 5:
                    a6_hi = 64 if last else max(0, min(done - 2, 64))
                    if a6_hi > st["a6"]:
                        nc.gpsimd.dma_start(
                            out=x6t[0:64, MOX + st["a6"] * PW: MOX + a6_hi * PW],
                            in_=xori[0:64, MO + st["a6"] * PW: MO + a6_hi * PW],
                        )
                        st["a6"] = a6_hi
                    b6_hi = 62 if last else max(0, min(done - 8, 62))
                    if b6_hi > st["b6"]:
                        nc.gpsimd.dma_start(
                            out=x6t[64:128, MOX + st["b6"] * PW: MOX + b6_hi * PW],
                            in_=xori[0:64, MO + (st["b6"] + 6) * PW: MO + (b6_hi + 6) * PW],
                        )
                        st["b6"] = b6_hi
                st["done"] = done

            def conv2_block(p, b):
                s = P[p]
                xori, x2t, x6t, outt, g3e = s["xori"], s["x2t"], s["x6t"], s["outt"], s["g3e"]
                sdt = s["sdt"]
                ps = ps13pool.tile([128, CHUNK], F32, name=f"ps2_{p}_{b}", tag="ps13")
                for kc in range(3):
                    dc = 2 * (kc - 1)
                    nc.tensor.matmul(
                        ps[0:128, 0:272],
                        w2dt[0:128, 128 * kc: 128 * (kc + 1)],
                        x2t[0:128, M2 + 4 * b * PW + dc: M2 + 4 * b * PW + dc + 272],
                        start=(kc == 0), stop=False,
                    )
                for kc in range(3):
                    dc = 2 * (kc - 1)
                    nc.tensor.matmul(
                        ps[0:128, 0:272],
                        w2ct[0:128, 128 * kc: 128 * (kc + 1)],
                        x6t[0:128, MOX + 4 * b * PW + dc: MOX + 4 * b * PW + dc + 272],
                        start=False, stop=(kc == 2),
                    )
                # gelu(conv2 + b2) for all 4 rows in one ACT, packed into the
                # interleaved output tile; then pull gelu3 where ~sd.
                ob = outt[0:128, 256 * b: 256 * (b + 1)].rearrange(
                    "a (r c) -> a r c", c=128)
                nc.scalar.activation(
                    ob,
                    ps[0:128, 0:272].rearrange(
                        "a (r c) -> a r c", c=PW)[:, :, 4:132],
                    act, bias=b2t[0:128, 0:1],
                )
                sb = sdt[0:128, 256 * b: 256 * (b + 1)].rearrange(
                    "a (r c) -> a r c", c=128)
                # both row-pair halves come from the packed g3 duplicate
                g3v = g3e[0:128, 0:32 * PW].rearrange("a (r c) -> a r c", c=PW)
                nc.vector.copy_predicated(
                    ob, sb, g3v[:, 2 * b: 2 * b + 2, 4:132],
                )
                if b % 4 == 3:
                    g = b // 4
                    nc.gpsimd.dma_start(
                        out=out_d.ap()[p, 0:64, 1024 * g: 1024 * (g + 1)],
                        in_=outt[0:64, 1024 * g: 1024 * (g + 1)],
                    )
                    nc.gpsimd.dma_start(
                        out=out_d.ap()[p, 0:64, 4096 + 1024 * g: 4096 + 1024 * (g + 1)],
                        in_=outt[64:128, 1024 * g: 1024 * (g + 1)],
                    )

            # software-pipelined driver: conv2 blocks trail their panel's
            # conv13 by one piece-gated group, so the PE never waits for the
            # full-panel pairing tiles; prev-panel late blocks fill the
            # conv13 ramp of the next panel
            setup_panel(0)
            emit_masks(0)
            emit_sd(0)
            for p in range(NP):
                if p == 0:
                    c13_group(0, 0, 2)
                    c13_group(0, 2, 8)
                else:
                    c13_group(p, 0, 8)
                if p > 0:
                    for b in range(9, NB):
                        conv2_block(p - 1, b)
                    emit_sd(p)
                if p + 1 < NP:
                    setup_panel(p + 1)
                c13_group(p, 8, 16)
                for b in range(0, 6):
                    conv2_block(p, b)
                c13_group(p, 16, 19)
                if p + 1 < NP:
                    emit_masks(p + 1)
                for b in range(6, 9):
                    conv2_block(p, b)
            for b in range(9, NB):
                conv2_block(NP - 1, b)

    nc.compile()
    _CACHE[key] = nc
    return nc


def _pack_weights(w1, w2, w3, b1, b2, b3):
    w13p = np.zeros((128, 3, 128), np.float32)
    w13s = np.zeros((64, 3, 128), np.float32)
    w2d = np.zeros((128, 3, 128), np.float32)
    w2c = np.zeros((128, 3, 128), np.float32)
    di = np.arange(64)
    for k in range(3):
        w13p[0:64, k, 0:64] = w1[:, :, 0, k].T
        w13p[64:128, k, 0:64] = w1[:, :, 1, k].T
        w13p[di, k, 64 + di] = w3[:, 0, 0, k]
        w13p[64 + di, k, 64 + di] = w3[:, 0, 1, k]

        w13s[0:64, k, 0:64] = w1[:, :, 2, k].T
        w13s[di, k, 64 + di] = w3[:, 0, 2, k]

        # conv2 pixel-pair: dense diag matmul covers 4 tap-applications
        w2d[0:64, k, 0:64] = w2[:, :, 1, k].T     # x(4b..) -> out(4b..)
        w2d[0:64, k, 64:128] = w2[:, :, 0, k].T   # x(4b..) -> out(4b+2..)
        w2d[64:128, k, 0:64] = w2[:, :, 2, k].T   # x(4b+2..) -> out(4b..)
        w2d[64:128, k, 64:128] = w2[:, :, 1, k].T
        # corner matmul via X6 (+6-shift pairing)
        w2c[0:64, k, 0:64] = w2[:, :, 0, k].T     # x(4b-2..) -> out(4b..)
        w2c[64:128, k, 64:128] = w2[:, :, 2, k].T  # x(4b+4..) -> out(4b+2..)
    return (
        np.ascontiguousarray(w13p.reshape(128, 384)),
        np.ascontiguousarray(w13s.reshape(64, 384)),
        np.ascontiguousarray(w2d.reshape(128, 384)),
        np.ascontiguousarray(w2c.reshape(128, 384)),
        np.concatenate([b1, b3]).reshape(128, 1).astype(np.float32),
        np.concatenate([b2, b2]).reshape(128, 1).astype(np.float32),
    )


def _dilate5(m):
    # 5x5 binary dilation, SAME/zero-pad semantics (max-pool)
    hh, ww = m.shape
    mp = np.pad(m, 2)
    a = np.maximum.reduce([mp[k: k + hh] for k in range(5)])      # [hh, ww+4]
    return np.maximum.reduce([a[:, k: k + ww] for k in range(5)])  # [hh, ww]


def make_in_maps(x, mask, w1, b1, w2, b2, w3, b3):
    import ml_dtypes
    BF = ml_dtypes.bfloat16
    x = np.asarray(x, np.float32)
    mask = np.asarray(mask, np.float32)

    w13p, w13s, w2d, w2c, b13p, b2dp = _pack_weights(
        np.asarray(w1, np.float32), np.asarray(w2, np.float32),
        np.asarray(w3, np.float32), np.asarray(b1, np.float32),
        np.asarray(b2, np.float32), np.asarray(b3, np.float32))
    w13p = w13p.astype(BF); w13s = w13s.astype(BF)
    w2d = w2d.astype(BF); w2c = w2c.astype(BF)

    xp32 = np.pad(x[0], ((0, 0), (4, 6), (4, 4)), mode="reflect")  # [64,522,520]
    xp = xp32.astype(BF)
    m = mask[0, 0]
    # inverted dilated mask: blend pulls x where ~md
    mdi = 1 - (_dilate5(m) > 0.5).astype(np.uint8)
    mdp = np.pad(mdi, ((2, 2), (4, 4)), mode="edge")   # [516,520]
    sdi = 1 - (m > 0.5).astype(np.uint8)               # inverted: pull gelu3

    ones = np.ones((64, PW), np.uint8)
    zeros = np.zeros((64, PW), np.uint8)

    in_maps = []
    for i in range(NCORES):
        r0 = RPC * i
        xc = np.empty((NP, 64, XROWS, PW), BF)
        mdc = np.empty((NP, 64, OROWS, PW), np.uint8)
        sdc = np.empty((NP, 128, NB, 2, PCOL), np.uint8)
        for p in range(NP):
            c0 = PCOL * p
            xc[p] = xp[:, r0: r0 + XROWS, c0: c0 + PW]
            mdc[p] = np.broadcast_to(
                mdp[r0: r0 + OROWS, c0: c0 + PW], (64, OROWS, PW))
            # packed interleaved ~sd: parts 0-63 = rows (4b,4b+1),
            # parts 64-127 = rows (4b+2,4b+3)
            sdi4 = sdi[r0: r0 + RPC, c0: c0 + PCOL].reshape(NB, 2, 2, PCOL)
            sdc[p, 0:64] = np.broadcast_to(sdi4[:, 0], (64, NB, 2, PCOL))
            sdc[p, 64:128] = np.broadcast_to(sdi4[:, 1], (64, NB, 2, PCOL))
        xc = np.ascontiguousarray(xc).reshape(NP, 64, XROWS * PW)
        mdc = np.ascontiguousarray(mdc).reshape(NP, 64, S13)
        sdc = np.ascontiguousarray(sdc).reshape(NP, 128, NB * 256)
        in_maps.append({
            "x_in": xc, "md_in": mdc, "sd_in": sdc,
            "w13p": w13p, "w13s": w13s, "w2d": w2d, "w2c": w2c,
            "b13": b13p, "b2dup": b2dp,
            "etopm": ones if i == 0 else zeros,
            "ebotm": ones if i == NCORES - 1 else zeros,
        })

    return in_maps


def kernel(x, mask, w1, b1, w2, b2, w3, b3):
    nc = _build_program()
    in_maps = make_in_maps(x, mask, w1, b1, w2, b2, w3, b3)
    global _last_in_maps
    _last_in_maps = in_maps
    res = run_bass_kernel_spmd(nc, in_maps, list(range(NCORES)))
    out = np.empty((1, C, H, W), np.float32)
    for i in range(NCORES):
        o = np.asarray(res.results[i]["out"], np.float32)  # [NP, 64, 8192] bf16
        # rows ordered [even-pairs | odd-pairs]: (half, b, rr) -> 4b+2*half+rr
        o = o.reshape(NP, C, 2, NB, 2, PCOL).transpose(1, 3, 2, 4, 0, 5)
        out[0, :, RPC * i: RPC * (i + 1), :] = o.reshape(C, RPC, W)
    out += np.asarray(x, np.float32).reshape(1, C, H, W)
    return out

